# revision 3
# baseline (speedup 1.0000x reference)
# DynamicPositionBias kernel for 8 Trainium2 NeuronCores.
#
# out[b, h, i, j] = qk[b, h, i, j] + table[i - j + N - 1, h]
# where table = MLP(pos) is a tiny (2N-1, H) bias table.
#
# Strategy:
#   * Host computes the (2N-1, H) table with numpy (negligible: ~16M flops).
#   * For each head, host builds a (128, 3968) f32 "master buffer" MB with
#     MB[p, c] = rev[c + 127 - p]  (rev = reversed table column), so the bias
#     for any 128-row stripe t of the (N, N) output is the SBUF view
#     MB[:, c0(t) : c0(t)+N] with c0(t) = 1920 - 128*t. One 2 MiB load per
#     head; zero per-tile bias traffic.
#   * Shard the 32 (b, h) slices head-paired: core c handles heads {2c, 2c+1}
#     for both batches, so only 2 master buffers per core.
#   * Device loop per core: load 4-stripe (128, 4, 2048) f32 blocks (4 MiB
#     DMA), add the bias views on VectorE in place, store. Loads on the SP
#     HWDGE ring, stores on the ACT HWDGE ring, triple-buffered via Tile.
import numpy as np

import concourse.bacc as bacc
import concourse.mybir as mybir
import concourse.tile as tile
from concourse.bass_utils import run_bass_kernel_spmd

_N = 2048
_H = 16
_B = 2
_NCORES = 8
_NSLICE = 4            # (b, h) slices per core
_HEADS_PER_CORE = 2
_R = 4                 # 128-row stripes per DMA block
_NT = _N // 128        # stripes per slice
_MBW = (2 * _N - 1) - 128 + 1  # 3968 master-buffer free size

_prog_cache = {}


def _build_program():
    if "nc" in _prog_cache:
        return _prog_cache["nc"]
    f32 = mybir.dt.float32
    nc = bacc.Bacc("TRN2", debug=False, target_bir_lowering=False,
                   num_devices=_NCORES)
    qk = nc.dram_tensor("qk", [_NSLICE, _N, _N], f32, kind="ExternalInput").ap()
    mb = nc.dram_tensor("mb", [_HEADS_PER_CORE, 128, _MBW], f32,
                        kind="ExternalInput").ap()
    out = nc.dram_tensor("out", [_NSLICE, _N, _N], f32,
                         kind="ExternalOutput").ap()

    with tile.TileContext(nc) as tc:
        with tc.tile_pool(name="mbp", bufs=2) as mbp, \
             tc.tile_pool(name="qkp", bufs=4) as qkp:
            mb_t = None
            for si in range(_NSLICE):
                if si % _HEADS_PER_CORE == 0:
                    mb_t = mbp.tile([128, _MBW], f32, name="mb_t")
                    nc.sync.dma_start(mb_t[:], mb[si // _HEADS_PER_CORE])
                qk_v = qk[si].rearrange("(t p) j -> p t j", p=128)
                out_v = out[si].rearrange("(t p) j -> p t j", p=128)
                for blk in range(_NT // _R):
                    t0 = blk * _R
                    qt = qkp.tile([128, _R, _N], f32, name="qt")
                    nc.sync.dma_start(qt[:], qk_v[:, t0:t0 + _R, :])
                    for r in range(_R):
                        c0 = (_MBW - _N) - 128 * (t0 + r)
                        nc.vector.tensor_add(qt[:, r, :], qt[:, r, :],
                                             mb_t[:, c0:c0 + _N])
                    nc.scalar.dma_start(out_v[:, t0:t0 + _R, :], qt[:])
    nc.compile()
    _prog_cache["nc"] = nc
    return nc


def _bias_table(W1, b1, W2, b2, W3, b3):
    pos = np.arange(-(_N - 1), _N, dtype=np.float32).reshape(-1, 1)
    h = np.maximum(pos @ W1 + b1, np.float32(0))
    h = np.maximum(h @ W2 + b2, np.float32(0))
    return h @ W3 + b3  # (2N-1, H) f32


def _master_buffers(table):
    # MB[h][p, c] = rev_h[c + 127 - p], rev_h[t] = table[2N-2-t, h]
    mbs = np.empty((_H, 128, _MBW), np.float32)
    for h in range(_H):
        rev = np.ascontiguousarray(table[::-1, h])
        swv = np.lib.stride_tricks.sliding_window_view(rev, _MBW)  # (128, MBW)
        mbs[h] = swv[::-1]
    return mbs


def _run(inputs, trace=False):
    qk = np.ascontiguousarray(np.asarray(inputs["qk_dots"], dtype=np.float32))
    table = _bias_table(
        np.asarray(inputs["W1"], np.float32), np.asarray(inputs["b1"], np.float32),
        np.asarray(inputs["W2"], np.float32), np.asarray(inputs["b2"], np.float32),
        np.asarray(inputs["W3"], np.float32), np.asarray(inputs["b3"], np.float32),
    )
    mbs = _master_buffers(table)

    in_maps = []
    for c in range(_NCORES):
        h0, h1 = 2 * c, 2 * c + 1
        qk_core = np.stack([qk[0, h0], qk[1, h0], qk[0, h1], qk[1, h1]])
        mb_core = np.stack([mbs[h0], mbs[h1]])
        in_maps.append({"qk": qk_core, "mb": mb_core})

    nc = _build_program()
    res = run_bass_kernel_spmd(nc, in_maps, list(range(_NCORES)), trace=trace)

    out = np.empty((_B, _H, _N, _N), np.float32)
    for c in range(_NCORES):
        o = res.results[c]["out"]
        for si in range(_NSLICE):
            out[si % 2, 2 * c + si // 2] = o[si]
    return out, res


def kernel(**inputs):
    assert tuple(np.shape(inputs["qk_dots"])) == (_B, _H, _N, _N)
    out, _ = _run(inputs)
    return out
